# revision 4
# baseline (speedup 1.0000x reference)
"""Trainium2 Bass kernel for 6-etype multi-head GAT (nn_GAT_4252017623767).

Strategy (8 NeuronCores, SPMD single NEFF):
  - Host: per etype, sort edges by dst; partition dst-blocks (128 rows) into
    per-core "instances" (block x padded tile count); build per-core compact
    node tables (union of needed src rows) and edge index streams.
  - Device phase A: z = x @ W (bf16), el/er = x @ (W@attn) packed as
    [z(128)bf16 | el(2)f32] rows in ztab, plus instance-ordered er table.
  - Device phase B: per 128-edge tile: indirect-gather z rows + er rows,
    ee = exp(leaky_relu(el+er)); one matmul per tile aggregates messages AND
    softmax denominators into a PSUM block via a 0/1 selection matrix built
    on-device with is_equal(rel, iota); per instance, normalize rows by the
    accumulated denominator and write the 128-dst-row block out once.
  - The rate-destination etype (10 dst rows) is edge-split across all cores;
    its raw partials go through one tiny AllReduce and are normalized at the
    end. Everything else needs no collectives: outputs are dst-sharded.
"""
import os
import sys

sys.path.insert(0, '/opt/trn_rl_repo')

import numpy as np
import ml_dtypes

import concourse.bass as bass
import concourse.bacc as bacc
import concourse.tile as tile
from concourse import mybir
from concourse.bass_utils import run_bass_kernel_spmd

bf16 = ml_dtypes.bfloat16
P = 128
ROW = 132          # z row: 128 bf16 + 2 f32 el (4 bf16 slots)
N_CORES = 8
G_DEFAULT = 16

ETYPES = [
    # (src_key, dst_key, src_table, dst_table)
    ('iu_src', 'iu_dst', 'item', 'user'),
    ('ui_src', 'ui_dst', 'user', 'item'),
    ('ic_src', 'ic_dst', 'item', 'cate'),
    ('ci_src', 'ci_dst', 'cate', 'item'),
    ('ir_src', 'ir_dst', 'item', 'rate'),
    ('ri_src', 'ri_dst', 'rate', 'item'),
]
TABLES = ['user', 'item', 'cate', 'rate']


def _choose_classes(ks, max_classes=3):
    """Pick <=max_classes tile-count class values (must cover max) minimizing
    total padded tiles. ks: array of per-block tile needs (>=1)."""
    uniq = np.unique(ks)
    best = None
    import itertools
    cand = list(uniq)
    for r in range(1, max_classes + 1):
        for combo in itertools.combinations(cand, r):
            if combo[-1] != uniq[-1]:
                continue
            arr = np.array(combo)
            idx = np.searchsorted(arr, ks)
            cost = int(arr[idx].sum())
            if best is None or cost < best[0]:
                best = (cost, arr)
    return best[1]


def host_prep(inputs, g=G_DEFAULT):
    sizes = {t: inputs[f'{t}_emb'].shape[0] for t in TABLES}
    toff = {}
    off = 0
    for t in TABLES:
        toff[t] = off
        off += sizes[t]
    NN = off
    x_cat = np.concatenate([np.asarray(inputs[f'{t}_emb']) for t in TABLES], axis=0)

    seg_off = {}
    off = 0
    for (sk, dk, st, dt) in ETYPES:
        seg_off[sk] = off
        off += sizes[dt]
    out_rows = off

    # ---- per-etype sort & instance construction ----
    # instance: dict(T, etype, block(global dst block base in dst-table), core?,
    #               src slice, rel slice, is_ir)
    per_core_inst = [[] for _ in range(N_CORES)]
    ir_slot_T = 0

    # pass 1: build instance lists per etype, assign to cores
    etype_insts = []  # (class_T, list of (etype_i, block, src_sorted slice, rel arr))
    for ei, (sk, dk, st, dt) in enumerate(ETYPES):
        src = np.asarray(inputs[sk])
        dst = np.asarray(inputs[dk])
        n_dst = sizes[dt]
        perm = np.argsort(dst, kind='stable')
        src_s = src[perm].astype(np.int64) + toff[st]
        dst_s = dst[perm].astype(np.int64)
        Bd = (n_dst + P - 1) // P
        if Bd < N_CORES:
            # split etype (rate dst): Bd must be 1
            assert Bd == 1
            n_tiles = (len(src_s) + P - 1) // P
            T_ir = (n_tiles + N_CORES - 1) // N_CORES
            ir_slot_T = T_ir
            for c in range(N_CORES):
                lo = min(c * T_ir * P, len(src_s))
                hi = min((c + 1) * T_ir * P, len(src_s))
                per_core_inst[c].append(dict(
                    T=T_ir, etype=ei, block=0, is_ir=True,
                    src=src_s[lo:hi], rel=dst_s[lo:hi].astype(np.float32)))
            continue
        blk = (dst_s // P).astype(np.int64)
        cnt = np.bincount(blk, minlength=Bd)
        ks = np.maximum(1, (cnt + P - 1) // P)
        classes = _choose_classes(ks)
        starts = np.concatenate([[0], np.cumsum(cnt)])
        cls_of = classes[np.searchsorted(classes, ks)]
        insts = []
        for b in range(Bd):
            insts.append(dict(
                T=int(cls_of[b]), etype=ei, block=b, is_ir=False,
                src=src_s[starts[b]:starts[b + 1]],
                rel=(dst_s[starts[b]:starts[b + 1]] - b * P).astype(np.float32)))
        # group instances by class; pad each class count to multiple of N_CORES
        for T in classes:
            cl = [i for i in insts if i['T'] == T]
            while len(cl) % N_CORES:
                cl.append(dict(T=int(T), etype=ei, block=-1, is_ir=False,
                               src=np.empty(0, np.int64),
                               rel=np.empty(0, np.float32)))
            # deal round-robin (sorted by edge count desc for mild balance)
            cl.sort(key=lambda d: -len(d['src']))
            for j, inst in enumerate(cl):
                per_core_inst[j % N_CORES].append(inst)

    # canonical slot order: ir first, then by (etype, T desc, block) — must be
    # IDENTICAL T-sequence across cores.
    for c in range(N_CORES):
        per_core_inst[c].sort(
            key=lambda d: (not d['is_ir'], d['etype'], -d['T'], d['block']))
    slot_Ts = [d['T'] for d in per_core_inst[0]]
    for c in range(1, N_CORES):
        assert [d['T'] for d in per_core_inst[c]] == slot_Ts, "non-uniform slots"
    NSLOT = len(slot_Ts)
    n_tiles = int(np.sum(slot_Ts))
    n_grp = (n_tiles + g - 1) // g

    # ---- per-core streams, compact tables ----
    cores = []
    NCs = []
    for c in range(N_CORES):
        insts = per_core_inst[c]
        srcflat = np.zeros(n_tiles * P, np.int64)
        relflat = np.full(n_tiles * P, -1.0, np.float32)
        slot_of_tile = np.repeat(np.arange(NSLOT), slot_Ts)
        tile_off = np.concatenate([[0], np.cumsum(slot_Ts)])
        dstids = np.zeros((NSLOT, P), np.int64)
        for s, inst in enumerate(insts):
            e0 = tile_off[s] * P
            cntr = len(inst['src'])
            srcflat[e0:e0 + cntr] = inst['src']
            relflat[e0:e0 + cntr] = inst['rel']
            if inst['block'] >= 0:
                sk, dk, st, dt = ETYPES[inst['etype']]
                base = toff[dt] + inst['block'] * P
                hi = toff[dt] + sizes[dt]
                dstids[s] = np.minimum(np.arange(base, base + P), hi - 1)
        real = relflat >= 0
        needed = np.unique(np.concatenate([srcflat[real], [0]]))
        srcix = np.zeros(n_tiles * P, np.int32)
        srcix[real] = np.searchsorted(needed, srcflat[real]).astype(np.int32)
        slot_per_edge = np.repeat(slot_of_tile, P)
        erix = (slot_per_edge * P + np.maximum(relflat, 0).astype(np.int64)
                ).astype(np.int32)
        cores.append(dict(insts=insts, srcix=srcix, relflat=relflat, erix=erix,
                          needed=needed, dstids=dstids))
        NCs.append(len(needed))
    NCpad = ((max(NCs) + P - 1) // P) * P

    def to_grp(a, dtp):
        full = np.zeros(n_grp * g * P, a.dtype)
        full[:n_tiles * P] = a
        if a.dtype == np.float32:
            full[n_tiles * P:] = -1.0
        return np.ascontiguousarray(
            full.reshape(n_grp, g, P).transpose(0, 2, 1)).astype(dtp)

    in_maps = []
    for c in range(N_CORES):
        d = cores[c]
        xt = np.zeros((P, NCpad), np.float32)
        xt[:, :NCs[c]] = x_cat[d['needed']].T
        xt_er = np.ascontiguousarray(x_cat[d['dstids'].reshape(-1)].T)
        W = np.asarray(inputs['W']).astype(np.float32)
        attn_l, attn_r = np.asarray(inputs['attn_l']), np.asarray(inputs['attn_r'])
        wcat = W.transpose(1, 0, 2).reshape(P, P)
        wT = W.transpose(0, 2, 1).reshape(P, P)
        attn = np.zeros((P, 2), np.float32)
        attn[0:64, 0] = attn_l[0]; attn[64:128, 0] = attn_l[1]
        attn[0:64, 1] = attn_r[0]; attn[64:128, 1] = attn_r[1]
        in_maps.append({
            'xt': xt.astype(bf16),
            'xt_er': xt_er.astype(bf16),
            'wcat': wcat.astype(bf16),
            'wT': wT.astype(np.float32),
            'attn': attn.astype(np.float32),
            'iota': np.broadcast_to(np.arange(P, dtype=np.float32), (P, P)).copy(),
            'src_g': to_grp(d['srcix'], np.int32),
            'rel_g': to_grp(d['relflat'], np.float32),
            'erix_g': to_grp(d['erix'], np.int32),
        })

    n_ir = sum(1 for d in per_core_inst[0] if d['is_ir'])
    ir_segs = [ETYPES[d['etype']][0] for d in per_core_inst[0] if d['is_ir']]
    meta = dict(NCpad=NCpad, NSLOT=NSLOT, slot_Ts=slot_Ts, n_tiles=n_tiles,
                n_grp=n_grp, G=g, out_rows=out_rows, seg_off=seg_off,
                sizes=sizes, n_ir=n_ir, ir_segs=ir_segs)
    return in_maps, cores, meta


def build_program(meta):
    NCpad, NSLOT = meta['NCpad'], meta['NSLOT']
    slot_Ts, n_grp, G = meta['slot_Ts'], meta['n_grp'], meta['G']
    n_tiles = meta['n_tiles']
    f32, bf, i32 = mybir.dt.float32, mybir.dt.bfloat16, mybir.dt.int32
    AF = mybir.ActivationFunctionType
    OP = mybir.AluOpType

    nc = bacc.Bacc("TRN2", target_bir_lowering=False, debug=False,
                   num_devices=N_CORES)
    xt_ap = nc.dram_tensor("xt", [P, NCpad], bf, kind="ExternalInput").ap()
    xter_ap = nc.dram_tensor("xt_er", [P, NSLOT * P], bf, kind="ExternalInput").ap()
    wcat_ap = nc.dram_tensor("wcat", [P, P], bf, kind="ExternalInput").ap()
    wT_ap = nc.dram_tensor("wT", [P, P], f32, kind="ExternalInput").ap()
    attn_ap = nc.dram_tensor("attn", [P, 2], f32, kind="ExternalInput").ap()
    iota_ap = nc.dram_tensor("iota", [P, P], f32, kind="ExternalInput").ap()
    srcg_ap = nc.dram_tensor("src_g", [n_grp, P, G], i32, kind="ExternalInput").ap()
    relg_ap = nc.dram_tensor("rel_g", [n_grp, P, G], f32, kind="ExternalInput").ap()
    erixg_ap = nc.dram_tensor("erix_g", [n_grp, P, G], i32, kind="ExternalInput").ap()
    stag_ap = nc.dram_tensor("stag", [NSLOT * P, P], f32, kind="ExternalOutput").ap()
    n_ir = meta['n_ir']
    stagir_ap = nc.dram_tensor("stag_ir", [max(n_ir, 1) * P, P], f32,
                               kind="ExternalOutput").ap()

    ztab = nc.dram_tensor("ztab", [NCpad, ROW], bf).ap()
    ertab = nc.dram_tensor("ertab", [NSLOT * P, 2], f32).ap()
    ir_ins = [nc.dram_tensor(f"ir_in{k}", [P, 130], f32) for k in range(n_ir)]
    ir_outs = [nc.dram_tensor(f"ir_out{k}", [P, 130], f32, addr_space="Shared")
               for k in range(n_ir)]

    with tile.TileContext(nc) as tc:
        with tc.tile_pool(name="cst", bufs=1) as cst, \
             tc.tile_pool(name="pa", bufs=3) as pa, \
             tc.tile_pool(name="ps_a", bufs=2, space="PSUM") as ps_a, \
             tc.tile_pool(name="sb", bufs=3) as sb, \
             tc.tile_pool(name="fl", bufs=4) as fl, \
             tc.tile_pool(name="ps", bufs=4, space="PSUM") as ps:
            # constants / weight prep
            iota_t = cst.tile([P, P], f32)
            nc.sync.dma_start(iota_t[:], iota_ap[:, :])
            wT_t = cst.tile([P, P], f32)
            nc.sync.dma_start(wT_t[:], wT_ap[:, :])
            attn_t = cst.tile([P, 2], f32)
            nc.sync.dma_start(attn_t[:], attn_ap[:, :])
            rhs_ext = cst.tile([P, ROW], bf)
            nc.sync.dma_start(rhs_ext[:, 0:P], wcat_ap[:, :])
            for h in range(2):
                for ci in range(2):
                    wl_ps = ps_a.tile([P, 1], f32, tag="zps")
                    nc.tensor.matmul(wl_ps[:], lhsT=wT_t[h * 64:(h + 1) * 64, :],
                                     rhs=attn_t[h * 64:(h + 1) * 64, ci:ci + 1],
                                     start=True, stop=True)
                    nc.vector.tensor_copy(
                        out=rhs_ext[:, P + 2 * ci + h:P + 2 * ci + h + 1],
                        in_=wl_ps[:])
            # phase A: compact nodes -> ztab
            for c in range(NCpad // P):
                xc = pa.tile([P, P], bf, tag="xc")
                nc.sync.dma_start(xc[:], xt_ap[:, c * P:(c + 1) * P])
                zps = ps_a.tile([P, ROW], f32, tag="zps")
                nc.tensor.matmul(zps[:], lhsT=xc[:], rhs=rhs_ext[:],
                                 start=True, stop=True)
                zrow = pa.tile([P, ROW], bf, tag="zrow")
                nc.vector.tensor_copy(out=zrow[:, 0:P], in_=zps[:, 0:P])
                nc.vector.tensor_copy(out=zrow[:, P:P + 4].bitcast(f32),
                                      in_=zps[:, P:P + 2])
                nc.sync.dma_start(ztab[c * P:(c + 1) * P, :], zrow[:])
            # phase A: er region (instance-ordered dst blocks)
            for i in range(NSLOT):
                xc = pa.tile([P, P], bf, tag="xc")
                nc.sync.dma_start(xc[:], xter_ap[:, i * P:(i + 1) * P])
                eps = ps_a.tile([P, 2], f32, tag="zps")
                nc.tensor.matmul(eps[:], lhsT=xc[:], rhs=rhs_ext[:, P + 2:P + 4],
                                 start=True, stop=True)
                ersb = pa.tile([P, 2], f32, tag="ersb")
                nc.vector.tensor_copy(out=ersb[:], in_=eps[:])
                nc.sync.dma_start(ertab[i * P:(i + 1) * P, :], ersb[:])

            tc.strict_bb_all_engine_barrier()

            # phase B
            slot_of_tile = np.repeat(np.arange(NSLOT), slot_Ts)
            tile_off = np.concatenate([[0], np.cumsum(slot_Ts)])
            psum_cur = None
            for grp in range(n_grp):
                t0 = grp * G
                gl = min(G, n_tiles - t0)
                if gl <= 0:
                    break
                srcix = sb.tile([P, G], i32, tag="srcix")
                relf = sb.tile([P, G], f32, tag="relf")
                erix_t = sb.tile([P, G], i32, tag="erix")
                nc.sync.dma_start(srcix[:], srcg_ap[grp])
                nc.sync.dma_start(relf[:], relg_ap[grp])
                nc.sync.dma_start(erix_t[:], erixg_ap[grp])
                zg = sb.tile([P, G, ROW], bf, tag="zg")
                erg = sb.tile([P, G, 2], f32, tag="erg")
                for gi in range(gl):
                    nc.gpsimd.indirect_dma_start(
                        out=zg[:, gi], out_offset=None, in_=ztab[:, :],
                        in_offset=bass.IndirectOffsetOnAxis(
                            ap=srcix[:, gi:gi + 1], axis=0))
                    nc.gpsimd.indirect_dma_start(
                        out=erg[:, gi], out_offset=None, in_=ertab[:, :],
                        in_offset=bass.IndirectOffsetOnAxis(
                            ap=erix_t[:, gi:gi + 1], axis=0))
                S = sb.tile([P, G, P], bf, tag="S")
                nc.vector.tensor_tensor(
                    out=S[:],
                    in0=relf[:].unsqueeze(2).to_broadcast([P, G, P]),
                    in1=iota_t[:].unsqueeze(1).to_broadcast([P, G, P]),
                    op=OP.is_equal)
                e_t = sb.tile([P, G, 2], f32, tag="e")
                nc.vector.tensor_tensor(out=e_t[:],
                                        in0=zg[:].bitcast(f32)[:, :, 64:66],
                                        in1=erg[:], op=OP.add)
                lk = sb.tile([P, G, 2], f32, tag="lk")
                nc.vector.tensor_scalar(out=lk[:], in0=e_t[:], scalar1=0.2,
                                        scalar2=None, op0=OP.mult)
                nc.vector.tensor_tensor(out=lk[:], in0=lk[:], in1=e_t[:],
                                        op=OP.max)
                ee = sb.tile([P, G, 2], f32, tag="ee")
                nc.scalar.activation(ee[:], lk[:], AF.Exp)
                msg = sb.tile([P, G, 130], bf, tag="msg")
                nc.vector.tensor_tensor(
                    out=msg[:, :, 0:P].rearrange("p g (h j) -> p g h j", h=2),
                    in0=zg[:, :, 0:P].rearrange("p g (h j) -> p g h j", h=2),
                    in1=ee[:].unsqueeze(3).to_broadcast([P, G, 2, 64]),
                    op=OP.mult)
                nc.vector.tensor_copy(out=msg[:, :, P:P + 2], in_=ee[:])
                for gi in range(gl):
                    t_ = t0 + gi
                    s = int(slot_of_tile[t_])
                    tt = t_ - int(tile_off[s])
                    T = slot_Ts[s]
                    if tt == 0:
                        psum_cur = ps.tile([P, 130], f32, tag="acc")
                    nc.tensor.matmul(psum_cur[:], lhsT=S[:, gi], rhs=msg[:, gi],
                                     start=(tt == 0), stop=(tt == T - 1))
                    if tt == T - 1:
                        if s < n_ir:
                            raw = fl.tile([P, 130], f32, tag="raw")
                            nc.vector.tensor_copy(out=raw[:], in_=psum_cur[:])
                            nc.sync.dma_start(ir_ins[s].ap()[:, :], raw[:])
                            nc.gpsimd.collective_compute(
                                "AllReduce", OP.add,
                                replica_groups=[list(range(N_CORES))],
                                ins=[ir_ins[s].ap().opt()],
                                outs=[ir_outs[s].ap().opt()])
                        else:
                            den = fl.tile([P, 2], f32, tag="den")
                            nc.vector.tensor_scalar(
                                out=den[:], in0=psum_cur[:, P:P + 2],
                                scalar1=1e-9, scalar2=None, op0=OP.add)
                            rec = fl.tile([P, 2], f32, tag="rec")
                            nc.vector.reciprocal(rec[:], den[:])
                            ot = fl.tile([P, P], f32, tag="ot")
                            for h in range(2):
                                nc.scalar.activation(
                                    ot[:, h * 64:(h + 1) * 64],
                                    psum_cur[:, h * 64:(h + 1) * 64],
                                    AF.Copy, scale=rec[:, h:h + 1])
                            nc.sync.dma_start(stag_ap[s * P:(s + 1) * P, :], ot[:])
            # ir tails: load AllReduced partials, normalize
            for k in range(n_ir):
                irt = fl.tile([P, 130], f32, tag="irt")
                nc.sync.dma_start(irt[:], ir_outs[k].ap()[:, :])
                den = fl.tile([P, 2], f32, tag="den")
                nc.vector.tensor_scalar(out=den[:], in0=irt[:, P:P + 2],
                                        scalar1=1e-9, scalar2=None, op0=OP.add)
                rec = fl.tile([P, 2], f32, tag="rec")
                nc.vector.reciprocal(rec[:], den[:])
                ot = fl.tile([P, P], f32, tag="ot")
                for h in range(2):
                    nc.scalar.activation(ot[:, h * 64:(h + 1) * 64],
                                         irt[:, h * 64:(h + 1) * 64],
                                         AF.Copy, scale=rec[:, h:h + 1])
                nc.sync.dma_start(stagir_ap[k * P:(k + 1) * P, :], ot[:])
    nc.compile()
    return nc


def assemble(results, cores, meta):
    out = np.zeros((meta['out_rows'], P), np.float32)
    sizes, seg_off = meta['sizes'], meta['seg_off']
    for c in range(N_CORES):
        stag = results[c]['stag']
        for s, inst in enumerate(cores[c]['insts']):
            if inst['is_ir'] or inst['block'] < 0:
                continue
            sk, dk, st, dt = ETYPES[inst['etype']]
            base = seg_off[sk] + inst['block'] * P
            n = min(P, seg_off[sk] + sizes[dt] - base)
            out[base:base + n] = stag[s * P:s * P + n]
    # split (single-dst-block) etypes from core 0's stag_ir
    for k, sk in enumerate(meta['ir_segs']):
        dt = [e[3] for e in ETYPES if e[0] == sk][0]
        out[seg_off[sk]:seg_off[sk] + sizes[dt]] = \
            results[0]['stag_ir'][k * P:k * P + sizes[dt]]
    return out


LAST_EXEC_NS = None
LAST_PROFILE = None


def kernel(**inputs):
    global LAST_EXEC_NS, LAST_PROFILE
    in_maps, cores, meta = host_prep(inputs)
    nc = build_program(meta)
    trace = os.environ.get('KERNEL_TRACE', '0') == '1'
    res = run_bass_kernel_spmd(nc, in_maps, core_ids=list(range(N_CORES)),
                               trace=trace)
    LAST_EXEC_NS = res.exec_time_ns
    LAST_PROFILE = res.profile_json
    return assemble(res.results, cores, meta)


# revision 6
# speedup vs baseline: 1.5246x; 1.5246x over previous
"""Trainium2 Bass kernel for 6-etype multi-head GAT (nn_GAT_4252017623767).

Strategy (8 NeuronCores, SPMD single NEFF):
  - Host: per etype, sort edges by dst; partition dst-blocks (128 rows) into
    per-core "instances" (block x padded tile count); build per-core compact
    node tables (union of needed src rows) and edge index streams.
  - Device phase A: z = x @ W (bf16), el/er = x @ (W@attn) packed as
    [z(128)bf16 | el(2)f32] rows in ztab, plus instance-ordered er table.
  - Device phase B: per 128-edge tile: indirect-gather z rows + er rows,
    ee = exp(leaky_relu(el+er)); one matmul per tile aggregates messages AND
    softmax denominators into a PSUM block via a 0/1 selection matrix built
    on-device with is_equal(rel, iota); per instance, normalize rows by the
    accumulated denominator and write the 128-dst-row block out once.
  - The rate-destination etype (10 dst rows) is edge-split across all cores;
    its raw partials go through one tiny AllReduce and are normalized at the
    end. Everything else needs no collectives: outputs are dst-sharded.
"""
import os
import sys

sys.path.insert(0, '/opt/trn_rl_repo')

import numpy as np
import ml_dtypes

import concourse.bass as bass
import concourse.bacc as bacc
import concourse.tile as tile
from concourse import mybir
from concourse.bass_utils import run_bass_kernel_spmd

bf16 = ml_dtypes.bfloat16
P = 128
ROW = 132          # z row: 128 bf16 + 2 f32 el (4 bf16 slots)
N_CORES = 8
G_DEFAULT = 16

ETYPES = [
    # (src_key, dst_key, src_table, dst_table)
    ('iu_src', 'iu_dst', 'item', 'user'),
    ('ui_src', 'ui_dst', 'user', 'item'),
    ('ic_src', 'ic_dst', 'item', 'cate'),
    ('ci_src', 'ci_dst', 'cate', 'item'),
    ('ir_src', 'ir_dst', 'item', 'rate'),
    ('ri_src', 'ri_dst', 'rate', 'item'),
]
TABLES = ['user', 'item', 'cate', 'rate']


def _choose_classes(ks, max_classes=3):
    """Pick <=max_classes tile-count class values (must cover max) minimizing
    total padded tiles. ks: array of per-block tile needs (>=1)."""
    uniq = np.unique(ks)
    best = None
    import itertools
    cand = list(uniq)
    for r in range(1, max_classes + 1):
        for combo in itertools.combinations(cand, r):
            if combo[-1] != uniq[-1]:
                continue
            arr = np.array(combo)
            idx = np.searchsorted(arr, ks)
            cost = int(arr[idx].sum())
            if best is None or cost < best[0]:
                best = (cost, arr)
    return best[1]


def host_prep(inputs, g=G_DEFAULT):
    sizes = {t: inputs[f'{t}_emb'].shape[0] for t in TABLES}
    toff = {}
    off = 0
    for t in TABLES:
        toff[t] = off
        off += sizes[t]
    NN = off
    x_cat = np.concatenate([np.asarray(inputs[f'{t}_emb']) for t in TABLES], axis=0)

    seg_off = {}
    off = 0
    for (sk, dk, st, dt) in ETYPES:
        seg_off[sk] = off
        off += sizes[dt]
    out_rows = off

    # ---- per-etype sort & instance construction ----
    # instance: dict(T, etype, block(global dst block base in dst-table), core?,
    #               src slice, rel slice, is_ir)
    per_core_inst = [[] for _ in range(N_CORES)]
    ir_slot_T = 0

    # pass 1: build instance lists per etype, assign to cores
    etype_insts = []  # (class_T, list of (etype_i, block, src_sorted slice, rel arr))
    for ei, (sk, dk, st, dt) in enumerate(ETYPES):
        src = np.asarray(inputs[sk])
        dst = np.asarray(inputs[dk])
        n_dst = sizes[dt]
        perm = np.argsort(dst, kind='stable')
        src_s = src[perm].astype(np.int64) + toff[st]
        dst_s = dst[perm].astype(np.int64)
        Bd = (n_dst + P - 1) // P
        if Bd < N_CORES:
            # split etype (rate dst): Bd must be 1
            assert Bd == 1
            n_tiles = (len(src_s) + P - 1) // P
            T_ir = (n_tiles + N_CORES - 1) // N_CORES
            ir_slot_T = T_ir
            for c in range(N_CORES):
                lo = min(c * T_ir * P, len(src_s))
                hi = min((c + 1) * T_ir * P, len(src_s))
                per_core_inst[c].append(dict(
                    T=T_ir, etype=ei, block=0, is_ir=True,
                    src=src_s[lo:hi], rel=dst_s[lo:hi].astype(np.float32)))
            continue
        blk = (dst_s // P).astype(np.int64)
        cnt = np.bincount(blk, minlength=Bd)
        ks = np.maximum(1, (cnt + P - 1) // P)
        classes = _choose_classes(ks)
        starts = np.concatenate([[0], np.cumsum(cnt)])
        cls_of = classes[np.searchsorted(classes, ks)]
        insts = []
        for b in range(Bd):
            insts.append(dict(
                T=int(cls_of[b]), etype=ei, block=b, is_ir=False,
                src=src_s[starts[b]:starts[b + 1]],
                rel=(dst_s[starts[b]:starts[b + 1]] - b * P).astype(np.float32)))
        # group instances by class; pad each class count to multiple of N_CORES
        for T in classes:
            cl = [i for i in insts if i['T'] == T]
            while len(cl) % N_CORES:
                cl.append(dict(T=int(T), etype=ei, block=-1, is_ir=False,
                               src=np.empty(0, np.int64),
                               rel=np.empty(0, np.float32)))
            # deal round-robin (sorted by edge count desc for mild balance)
            cl.sort(key=lambda d: -len(d['src']))
            for j, inst in enumerate(cl):
                per_core_inst[j % N_CORES].append(inst)

    # canonical slot order: ir first, then by (etype, T desc, block) — must be
    # IDENTICAL T-sequence across cores.
    for c in range(N_CORES):
        per_core_inst[c].sort(
            key=lambda d: (not d['is_ir'], d['etype'], -d['T'], d['block']))
    slot_Ts = [d['T'] for d in per_core_inst[0]]
    for c in range(1, N_CORES):
        assert [d['T'] for d in per_core_inst[c]] == slot_Ts, "non-uniform slots"
    NSLOT = len(slot_Ts)
    n_tiles = int(np.sum(slot_Ts))
    n_grp = (n_tiles + g - 1) // g

    # ---- per-core streams, compact tables ----
    cores = []
    NCs = []
    for c in range(N_CORES):
        insts = per_core_inst[c]
        srcflat = np.zeros(n_tiles * P, np.int64)
        relflat = np.full(n_tiles * P, -1.0, np.float32)
        slot_of_tile = np.repeat(np.arange(NSLOT), slot_Ts)
        tile_off = np.concatenate([[0], np.cumsum(slot_Ts)])
        dstids = np.zeros((NSLOT, P), np.int64)
        for s, inst in enumerate(insts):
            e0 = tile_off[s] * P
            cntr = len(inst['src'])
            srcflat[e0:e0 + cntr] = inst['src']
            relflat[e0:e0 + cntr] = inst['rel']
            if inst['block'] >= 0:
                sk, dk, st, dt = ETYPES[inst['etype']]
                base = toff[dt] + inst['block'] * P
                hi = toff[dt] + sizes[dt]
                dstids[s] = np.minimum(np.arange(base, base + P), hi - 1)
        real = relflat >= 0
        needed = np.unique(np.concatenate([srcflat[real], [0]]))
        srcix = np.zeros(n_tiles * P, np.int32)
        srcix[real] = np.searchsorted(needed, srcflat[real]).astype(np.int32)
        slot_per_edge = np.repeat(slot_of_tile, P)
        erix = (slot_per_edge * P + np.maximum(relflat, 0).astype(np.int64)
                ).astype(np.int32)
        cores.append(dict(insts=insts, srcix=srcix, relflat=relflat, erix=erix,
                          needed=needed, dstids=dstids))
        NCs.append(len(needed))
    NCpad = ((max(NCs) + P - 1) // P) * P

    def to_grp(a, dtp):
        full = np.zeros(n_grp * g * P, a.dtype)
        full[:n_tiles * P] = a
        if a.dtype == np.float32:
            full[n_tiles * P:] = -1.0
        return np.ascontiguousarray(
            full.reshape(n_grp, g, P).transpose(0, 2, 1)).astype(dtp)

    def to_relT(a):
        full = np.full(n_grp * g * P, -1.0, np.float32)
        full[:n_tiles * P] = a
        return np.ascontiguousarray(full.reshape(n_grp, 1, g * P))

    in_maps = []
    for c in range(N_CORES):
        d = cores[c]
        xt = np.zeros((P, NCpad), np.float32)
        xt[:, :NCs[c]] = x_cat[d['needed']].T
        xt_er = np.ascontiguousarray(x_cat[d['dstids'].reshape(-1)].T)
        W = np.asarray(inputs['W']).astype(np.float32)
        attn_l, attn_r = np.asarray(inputs['attn_l']), np.asarray(inputs['attn_r'])
        wcat = W.transpose(1, 0, 2).reshape(P, P)
        wT = W.transpose(0, 2, 1).reshape(P, P)
        attn = np.zeros((P, 2), np.float32)
        attn[0:64, 0] = attn_l[0]; attn[64:128, 0] = attn_l[1]
        attn[0:64, 1] = attn_r[0]; attn[64:128, 1] = attn_r[1]
        in_maps.append({
            'xt': xt.astype(bf16),
            'xt_er': xt_er.astype(bf16),
            'wcat': wcat.astype(bf16),
            'wT': wT.astype(np.float32),
            'attn': attn.astype(np.float32),
            'iota': np.broadcast_to(np.arange(P, dtype=np.float32), (P, P)).copy(),
            'src_g': to_grp(d['srcix'], np.int32),
            'rel_g': to_grp(d['relflat'], np.float32),
            'relT': to_relT(d['relflat']),
            'iota_p': np.arange(P, dtype=np.float32).reshape(P, 1).astype(bf16),
            'ones': np.ones((1, P), np.float32),
        })

    n_ir = sum(1 for d in per_core_inst[0] if d['is_ir'])
    ir_segs = [ETYPES[d['etype']][0] for d in per_core_inst[0] if d['is_ir']]
    meta = dict(NCpad=NCpad, NSLOT=NSLOT, slot_Ts=slot_Ts, n_tiles=n_tiles,
                n_grp=n_grp, G=g, out_rows=out_rows, seg_off=seg_off,
                sizes=sizes, n_ir=n_ir, ir_segs=ir_segs)
    return in_maps, cores, meta


def build_program(meta):
    NCpad, NSLOT = meta['NCpad'], meta['NSLOT']
    slot_Ts, n_grp, G = meta['slot_Ts'], meta['n_grp'], meta['G']
    n_tiles = meta['n_tiles']
    f32, bf, i32 = mybir.dt.float32, mybir.dt.bfloat16, mybir.dt.int32
    AF = mybir.ActivationFunctionType
    OP = mybir.AluOpType

    nc = bacc.Bacc("TRN2", target_bir_lowering=False, debug=False,
                   num_devices=N_CORES)
    xt_ap = nc.dram_tensor("xt", [P, NCpad], bf, kind="ExternalInput").ap()
    xter_ap = nc.dram_tensor("xt_er", [P, NSLOT * P], bf, kind="ExternalInput").ap()
    wcat_ap = nc.dram_tensor("wcat", [P, P], bf, kind="ExternalInput").ap()
    wT_ap = nc.dram_tensor("wT", [P, P], f32, kind="ExternalInput").ap()
    attn_ap = nc.dram_tensor("attn", [P, 2], f32, kind="ExternalInput").ap()
    iota_ap = nc.dram_tensor("iota", [P, P], f32, kind="ExternalInput").ap()
    srcg_ap = nc.dram_tensor("src_g", [n_grp, P, G], i32, kind="ExternalInput").ap()
    relg_ap = nc.dram_tensor("rel_g", [n_grp, P, G], f32, kind="ExternalInput").ap()
    relT_ap = nc.dram_tensor("relT", [n_grp, 1, G * P], f32, kind="ExternalInput").ap()
    iop_ap = nc.dram_tensor("iota_p", [P, 1], bf, kind="ExternalInput").ap()
    ones_ap = nc.dram_tensor("ones", [1, P], f32, kind="ExternalInput").ap()
    stag_ap = nc.dram_tensor("stag", [NSLOT * P, P], f32, kind="ExternalOutput").ap()
    n_ir = meta['n_ir']
    stagir_ap = nc.dram_tensor("stag_ir", [max(n_ir, 1) * P, P], f32,
                               kind="ExternalOutput").ap()

    ztab = nc.dram_tensor("ztab", [NCpad, ROW], bf).ap()
    ertab = nc.dram_tensor("ertab", [NSLOT * P, 2], f32).ap()
    ir_ins = [nc.dram_tensor(f"ir_in{k}", [P, 130], f32) for k in range(n_ir)]
    ir_outs = [nc.dram_tensor(f"ir_out{k}", [P, 130], f32, addr_space="Shared")
               for k in range(n_ir)]

    with tile.TileContext(nc) as tc:
        with tc.tile_pool(name="cst", bufs=1) as cst, \
             tc.tile_pool(name="pa", bufs=3) as pa, \
             tc.tile_pool(name="ps_a", bufs=1, space="PSUM") as ps_a, \
             tc.tile_pool(name="sb", bufs=3) as sb, \
             tc.tile_pool(name="fl", bufs=4) as fl, \
             tc.tile_pool(name="ps", bufs=3, space="PSUM") as ps, \
             tc.tile_pool(name="psr", bufs=2, space="PSUM") as psr, \
             tc.tile_pool(name="pse", bufs=2, space="PSUM") as pse:
            # constants / weight prep
            iota_t = cst.tile([P, P], f32)
            nc.sync.dma_start(iota_t[:], iota_ap[:, :])
            iop_t = cst.tile([P, 1], bf)
            nc.sync.dma_start(iop_t[:], iop_ap[:, :])
            ones_t = cst.tile([1, P], f32)
            nc.sync.dma_start(ones_t[:], ones_ap[:, :])
            wT_t = cst.tile([P, P], f32)
            nc.sync.dma_start(wT_t[:], wT_ap[:, :])
            attn_t = cst.tile([P, 2], f32)
            nc.sync.dma_start(attn_t[:], attn_ap[:, :])
            rhs_ext = cst.tile([P, ROW], bf)
            nc.sync.dma_start(rhs_ext[:, 0:P], wcat_ap[:, :])
            for h in range(2):
                for ci in range(2):
                    wl_ps = ps_a.tile([P, 1], f32, tag="zps")
                    nc.tensor.matmul(wl_ps[:], lhsT=wT_t[h * 64:(h + 1) * 64, :],
                                     rhs=attn_t[h * 64:(h + 1) * 64, ci:ci + 1],
                                     start=True, stop=True)
                    nc.vector.tensor_copy(
                        out=rhs_ext[:, P + 2 * ci + h:P + 2 * ci + h + 1],
                        in_=wl_ps[:])
            # phase A: compact nodes -> ztab
            for c in range(NCpad // P):
                xc = pa.tile([P, P], bf, tag="xc")
                nc.sync.dma_start(xc[:], xt_ap[:, c * P:(c + 1) * P])
                zps = ps_a.tile([P, ROW], f32, tag="zps")
                nc.tensor.matmul(zps[:], lhsT=xc[:], rhs=rhs_ext[:],
                                 start=True, stop=True)
                zrow = pa.tile([P, ROW], bf, tag="zrow")
                nc.vector.tensor_copy(out=zrow[:, 0:P], in_=zps[:, 0:P])
                nc.vector.tensor_copy(out=zrow[:, P:P + 4].bitcast(f32),
                                      in_=zps[:, P:P + 2])
                nc.sync.dma_start(ztab[c * P:(c + 1) * P, :], zrow[:])
            # phase A: er region (instance-ordered dst blocks)
            for i in range(NSLOT):
                xc = pa.tile([P, P], bf, tag="xc")
                nc.sync.dma_start(xc[:], xter_ap[:, i * P:(i + 1) * P])
                eps = ps_a.tile([P, 2], f32, tag="zps")
                nc.tensor.matmul(eps[:], lhsT=xc[:], rhs=rhs_ext[:, P + 2:P + 4],
                                 start=True, stop=True)
                ersb = pa.tile([P, 2], f32, tag="ersb")
                nc.vector.tensor_copy(out=ersb[:], in_=eps[:])
                nc.sync.dma_start(ertab[i * P:(i + 1) * P, :], ersb[:])

            tc.strict_bb_all_engine_barrier()

            # phase B
            slot_of_tile = np.repeat(np.arange(NSLOT), slot_Ts)
            tile_off = np.concatenate([[0], np.cumsum(slot_Ts)])
            psum_cur = None
            for grp in range(n_grp):
                t0 = grp * G
                gl = min(G, n_tiles - t0)
                if gl <= 0:
                    break
                srcix = sb.tile([P, G], i32, tag="srcix")
                relf = sb.tile([P, G], f32, tag="relf")
                relT_t = sb.tile([1, G * P], f32, tag="relT")
                nc.sync.dma_start(srcix[:], srcg_ap[grp])
                nc.sync.dma_start(relf[:], relg_ap[grp])
                nc.sync.dma_start(relT_t[:], relT_ap[grp])
                zg = sb.tile([P, G, ROW], bf, tag="zg")
                erg = sb.tile([P, G, 2], f32, tag="erg")
                for gi in range(gl):
                    nc.gpsimd.indirect_dma_start(
                        out=zg[:, gi], out_offset=None, in_=ztab[:, :],
                        in_offset=bass.IndirectOffsetOnAxis(
                            ap=srcix[:, gi:gi + 1], axis=0))
                # replicate relT across partitions (ones-matmul), cast to bf16
                relrep = sb.tile([P, G * P], bf, tag="relrep")
                for k in range((G * P + 511) // 512):
                    c0, c1 = k * 512, min((k + 1) * 512, G * P)
                    rps = psr.tile([P, 512], f32, tag="rps")
                    nc.tensor.matmul(rps[:, :c1 - c0], lhsT=ones_t[:],
                                     rhs=relT_t[:, c0:c1], start=True, stop=True)
                    nc.scalar.activation(relrep[:, c0:c1], rps[:, :c1 - c0],
                                         AF.Copy)
                S_T = sb.tile([P, G * P], bf, tag="S_T")
                nc.vector.tensor_tensor(
                    out=S_T[:],
                    in0=iop_t[:].to_broadcast([P, G * P]),
                    in1=relrep[:],
                    op=OP.is_equal)
                for gi in range(gl):
                    t_ = t0 + gi
                    s = int(slot_of_tile[t_])
                    tt = t_ - int(tile_off[s])
                    if tt == 0:
                        erblk = fl.tile([P, 2], bf, tag="erblk")
                        erblk_f = fl.tile([P, 2], f32, tag="erblkf")
                        nc.sync.dma_start(erblk_f[:], ertab[s * P:(s + 1) * P, :])
                        nc.vector.tensor_copy(out=erblk[:], in_=erblk_f[:])
                        cur_erblk = erblk
                    eps_t = pse.tile([P, 2], f32, tag="eps")
                    nc.tensor.matmul(eps_t[:], lhsT=S_T[:, gi * P:(gi + 1) * P],
                                     rhs=cur_erblk[:], start=True, stop=True)
                    nc.vector.tensor_copy(out=erg[:, gi], in_=eps_t[:])
                S = sb.tile([P, G, P], bf, tag="S")
                nc.vector.tensor_tensor(
                    out=S[:],
                    in0=relf[:].unsqueeze(2).to_broadcast([P, G, P]),
                    in1=iota_t[:].unsqueeze(1).to_broadcast([P, G, P]),
                    op=OP.is_equal)
                e_t = sb.tile([P, G, 2], f32, tag="e")
                nc.vector.tensor_tensor(out=e_t[:],
                                        in0=zg[:].bitcast(f32)[:, :, 64:66],
                                        in1=erg[:], op=OP.add)
                lk = sb.tile([P, G, 2], f32, tag="lk")
                nc.vector.tensor_scalar(out=lk[:], in0=e_t[:], scalar1=0.2,
                                        scalar2=None, op0=OP.mult)
                nc.vector.tensor_tensor(out=lk[:], in0=lk[:], in1=e_t[:],
                                        op=OP.max)
                ee = sb.tile([P, G, 2], f32, tag="ee")
                nc.scalar.activation(ee[:], lk[:], AF.Exp)
                msg = sb.tile([P, G, 130], bf, tag="msg")
                nc.vector.tensor_tensor(
                    out=msg[:, :, 0:P].rearrange("p g (h j) -> p g h j", h=2),
                    in0=zg[:, :, 0:P].rearrange("p g (h j) -> p g h j", h=2),
                    in1=ee[:].unsqueeze(3).to_broadcast([P, G, 2, 64]),
                    op=OP.mult)
                nc.vector.tensor_copy(out=msg[:, :, P:P + 2], in_=ee[:])
                for gi in range(gl):
                    t_ = t0 + gi
                    s = int(slot_of_tile[t_])
                    tt = t_ - int(tile_off[s])
                    T = slot_Ts[s]
                    if tt == 0:
                        psum_cur = ps.tile([P, 130], f32, tag="acc")
                    nc.tensor.matmul(psum_cur[:], lhsT=S[:, gi], rhs=msg[:, gi],
                                     start=(tt == 0), stop=(tt == T - 1))
                    if tt == T - 1:
                        if s < n_ir:
                            raw = fl.tile([P, 130], f32, tag="raw")
                            nc.vector.tensor_copy(out=raw[:], in_=psum_cur[:])
                            nc.sync.dma_start(ir_ins[s].ap()[:, :], raw[:])
                            nc.gpsimd.collective_compute(
                                "AllReduce", OP.add,
                                replica_groups=[list(range(N_CORES))],
                                ins=[ir_ins[s].ap().opt()],
                                outs=[ir_outs[s].ap().opt()])
                        else:
                            den = fl.tile([P, 2], f32, tag="den")
                            nc.vector.tensor_scalar(
                                out=den[:], in0=psum_cur[:, P:P + 2],
                                scalar1=1e-9, scalar2=None, op0=OP.add)
                            rec = fl.tile([P, 2], f32, tag="rec")
                            nc.vector.reciprocal(rec[:], den[:])
                            ot = fl.tile([P, P], f32, tag="ot")
                            for h in range(2):
                                nc.scalar.activation(
                                    ot[:, h * 64:(h + 1) * 64],
                                    psum_cur[:, h * 64:(h + 1) * 64],
                                    AF.Copy, scale=rec[:, h:h + 1])
                            nc.sync.dma_start(stag_ap[s * P:(s + 1) * P, :], ot[:])
            # ir tails: load AllReduced partials, normalize
            for k in range(n_ir):
                irt = fl.tile([P, 130], f32, tag="irt")
                nc.sync.dma_start(irt[:], ir_outs[k].ap()[:, :])
                den = fl.tile([P, 2], f32, tag="den")
                nc.vector.tensor_scalar(out=den[:], in0=irt[:, P:P + 2],
                                        scalar1=1e-9, scalar2=None, op0=OP.add)
                rec = fl.tile([P, 2], f32, tag="rec")
                nc.vector.reciprocal(rec[:], den[:])
                ot = fl.tile([P, P], f32, tag="ot")
                for h in range(2):
                    nc.scalar.activation(ot[:, h * 64:(h + 1) * 64],
                                         irt[:, h * 64:(h + 1) * 64],
                                         AF.Copy, scale=rec[:, h:h + 1])
                nc.sync.dma_start(stagir_ap[k * P:(k + 1) * P, :], ot[:])
    nc.compile()
    return nc


def assemble(results, cores, meta):
    out = np.zeros((meta['out_rows'], P), np.float32)
    sizes, seg_off = meta['sizes'], meta['seg_off']
    for c in range(N_CORES):
        stag = results[c]['stag']
        for s, inst in enumerate(cores[c]['insts']):
            if inst['is_ir'] or inst['block'] < 0:
                continue
            sk, dk, st, dt = ETYPES[inst['etype']]
            base = seg_off[sk] + inst['block'] * P
            n = min(P, seg_off[sk] + sizes[dt] - base)
            out[base:base + n] = stag[s * P:s * P + n]
    # split (single-dst-block) etypes from core 0's stag_ir
    for k, sk in enumerate(meta['ir_segs']):
        dt = [e[3] for e in ETYPES if e[0] == sk][0]
        out[seg_off[sk]:seg_off[sk] + sizes[dt]] = \
            results[0]['stag_ir'][k * P:k * P + sizes[dt]]
    return out


LAST_EXEC_NS = None
LAST_PROFILE = None


def kernel(**inputs):
    global LAST_EXEC_NS, LAST_PROFILE
    in_maps, cores, meta = host_prep(inputs)
    nc = build_program(meta)
    trace = os.environ.get('KERNEL_TRACE', '0') == '1'
    res = run_bass_kernel_spmd(nc, in_maps, core_ids=list(range(N_CORES)),
                               trace=trace)
    LAST_EXEC_NS = res.exec_time_ns
    LAST_PROFILE = res.profile_json
    return assemble(res.results, cores, meta)


# revision 9
# speedup vs baseline: 1.9453x; 1.2760x over previous
"""Trainium2 Bass kernel for 6-etype multi-head GAT (nn_GAT_4252017623767).

Strategy (8 NeuronCores, SPMD single NEFF):
  - Host: per etype, sort edges by dst; partition dst-blocks (128 rows) into
    per-core "instances" (block x padded tile count); build per-core compact
    node tables (union of needed src rows) and edge index streams.
  - Device phase A: z = x @ W (bf16), el/er = x @ (W@attn) packed as
    [z(128)bf16 | el(2)f32] rows in ztab, plus instance-ordered er table.
  - Device phase B: per 128-edge tile: indirect-gather z rows + er rows,
    ee = exp(leaky_relu(el+er)); one matmul per tile aggregates messages AND
    softmax denominators into a PSUM block via a 0/1 selection matrix built
    on-device with is_equal(rel, iota); per instance, normalize rows by the
    accumulated denominator and write the 128-dst-row block out once.
  - The rate-destination etype (10 dst rows) is edge-split across all cores;
    its raw partials go through one tiny AllReduce and are normalized at the
    end. Everything else needs no collectives: outputs are dst-sharded.
"""
import os
import sys

sys.path.insert(0, '/opt/trn_rl_repo')

import numpy as np
import ml_dtypes

import concourse.bass as bass
import concourse.bacc as bacc
import concourse.tile as tile
from concourse import mybir
from concourse.bass_utils import run_bass_kernel_spmd

bf16 = ml_dtypes.bfloat16
P = 128
ROW = 132          # z row: 128 bf16 + 2 f32 el (4 bf16 slots)
N_CORES = 8
G_DEFAULT = 16

ETYPES = [
    # (src_key, dst_key, src_table, dst_table)
    ('iu_src', 'iu_dst', 'item', 'user'),
    ('ui_src', 'ui_dst', 'user', 'item'),
    ('ic_src', 'ic_dst', 'item', 'cate'),
    ('ci_src', 'ci_dst', 'cate', 'item'),
    ('ir_src', 'ir_dst', 'item', 'rate'),
    ('ri_src', 'ri_dst', 'rate', 'item'),
]
TABLES = ['user', 'item', 'cate', 'rate']


def _choose_classes(ks, max_classes=3):
    """Pick <=max_classes tile-count class values (must cover max) minimizing
    total padded tiles. ks: array of per-block tile needs (>=1)."""
    uniq = np.unique(ks)
    best = None
    import itertools
    cand = list(uniq)
    for r in range(1, max_classes + 1):
        for combo in itertools.combinations(cand, r):
            if combo[-1] != uniq[-1]:
                continue
            arr = np.array(combo)
            idx = np.searchsorted(arr, ks)
            cost = int(arr[idx].sum())
            if best is None or cost < best[0]:
                best = (cost, arr)
    return best[1]


def host_prep(inputs, g=G_DEFAULT):
    sizes = {t: inputs[f'{t}_emb'].shape[0] for t in TABLES}
    toff = {}
    off = 0
    for t in TABLES:
        toff[t] = off
        off += sizes[t]
    NN = off
    x_cat = np.concatenate([np.asarray(inputs[f'{t}_emb']) for t in TABLES], axis=0)

    seg_off = {}
    off = 0
    for (sk, dk, st, dt) in ETYPES:
        seg_off[sk] = off
        off += sizes[dt]
    out_rows = off

    # ---- per-etype sort & instance construction ----
    # instance: dict(T, etype, block(global dst block base in dst-table), core?,
    #               src slice, rel slice, is_ir)
    per_core_inst = [[] for _ in range(N_CORES)]
    ir_slot_T = 0

    # pass 1: build instance lists per etype, assign to cores
    etype_insts = []  # (class_T, list of (etype_i, block, src_sorted slice, rel arr))
    for ei, (sk, dk, st, dt) in enumerate(ETYPES):
        src = np.asarray(inputs[sk])
        dst = np.asarray(inputs[dk])
        n_dst = sizes[dt]
        perm = np.argsort(dst, kind='stable')
        src_s = src[perm].astype(np.int64) + toff[st]
        dst_s = dst[perm].astype(np.int64)
        Bd = (n_dst + P - 1) // P
        if Bd < N_CORES:
            # split etype (rate dst): Bd must be 1
            assert Bd == 1
            n_tiles = (len(src_s) + P - 1) // P
            T_ir = (n_tiles + N_CORES - 1) // N_CORES
            ir_slot_T = T_ir
            for c in range(N_CORES):
                lo = min(c * T_ir * P, len(src_s))
                hi = min((c + 1) * T_ir * P, len(src_s))
                per_core_inst[c].append(dict(
                    T=T_ir, etype=ei, block=0, is_ir=True,
                    src=src_s[lo:hi], rel=dst_s[lo:hi].astype(np.float32)))
            continue
        blk = (dst_s // P).astype(np.int64)
        cnt = np.bincount(blk, minlength=Bd)
        ks = np.maximum(1, (cnt + P - 1) // P)
        classes = _choose_classes(ks)
        starts = np.concatenate([[0], np.cumsum(cnt)])
        cls_of = classes[np.searchsorted(classes, ks)]
        insts = []
        for b in range(Bd):
            insts.append(dict(
                T=int(cls_of[b]), etype=ei, block=b, is_ir=False,
                src=src_s[starts[b]:starts[b + 1]],
                rel=(dst_s[starts[b]:starts[b + 1]] - b * P).astype(np.float32)))
        # group instances by class; pad each class count to multiple of N_CORES
        for T in classes:
            cl = [i for i in insts if i['T'] == T]
            while len(cl) % N_CORES:
                cl.append(dict(T=int(T), etype=ei, block=-1, is_ir=False,
                               src=np.empty(0, np.int64),
                               rel=np.empty(0, np.float32)))
            # deal round-robin (sorted by edge count desc for mild balance)
            cl.sort(key=lambda d: -len(d['src']))
            for j, inst in enumerate(cl):
                per_core_inst[j % N_CORES].append(inst)

    # canonical slot order: ir first, then by (etype, T desc, block) — must be
    # IDENTICAL T-sequence across cores.
    for c in range(N_CORES):
        per_core_inst[c].sort(
            key=lambda d: (not d['is_ir'], d['etype'], -d['T'], d['block']))
    slot_Ts = [d['T'] for d in per_core_inst[0]]
    for c in range(1, N_CORES):
        assert [d['T'] for d in per_core_inst[c]] == slot_Ts, "non-uniform slots"
    NSLOT = len(slot_Ts)
    n_tiles = int(np.sum(slot_Ts))
    n_grp = (n_tiles + g - 1) // g

    # ---- per-core streams, compact tables ----
    cores = []
    NCs = []
    for c in range(N_CORES):
        insts = per_core_inst[c]
        srcflat = np.zeros(n_tiles * P, np.int64)
        relflat = np.full(n_tiles * P, -1.0, np.float32)
        slot_of_tile = np.repeat(np.arange(NSLOT), slot_Ts)
        tile_off = np.concatenate([[0], np.cumsum(slot_Ts)])
        dstids = np.zeros((NSLOT, P), np.int64)
        for s, inst in enumerate(insts):
            e0 = tile_off[s] * P
            cntr = len(inst['src'])
            srcflat[e0:e0 + cntr] = inst['src']
            relflat[e0:e0 + cntr] = inst['rel']
            if inst['block'] >= 0:
                sk, dk, st, dt = ETYPES[inst['etype']]
                base = toff[dt] + inst['block'] * P
                hi = toff[dt] + sizes[dt]
                dstids[s] = np.minimum(np.arange(base, base + P), hi - 1)
        real = relflat >= 0
        needed = np.unique(np.concatenate([srcflat[real], [0]]))
        srcix = np.zeros(n_tiles * P, np.int32)
        srcix[real] = np.searchsorted(needed, srcflat[real]).astype(np.int32)
        slot_per_edge = np.repeat(slot_of_tile, P)
        erix = (slot_per_edge * P + np.maximum(relflat, 0).astype(np.int64)
                ).astype(np.int32)
        cores.append(dict(insts=insts, srcix=srcix, relflat=relflat, erix=erix,
                          needed=needed, dstids=dstids))
        NCs.append(len(needed))
    NCpad = ((max(NCs) + 4 * P - 1) // (4 * P)) * (4 * P)

    def to_grp(a, dtp):
        full = np.zeros(n_grp * g * P, a.dtype)
        full[:n_tiles * P] = a
        if a.dtype == np.float32:
            full[n_tiles * P:] = -1.0
        return np.ascontiguousarray(
            full.reshape(n_grp, g, P).transpose(0, 2, 1)).astype(dtp)

    def to_relT(a):
        full = np.full(n_grp * g * P, -1.0, np.float32)
        full[:n_tiles * P] = a
        return np.ascontiguousarray(full.reshape(n_grp, 1, g * P))

    in_maps = []
    for c in range(N_CORES):
        d = cores[c]
        xt = np.zeros((P, NCpad), np.float32)
        xt[:, :NCs[c]] = x_cat[d['needed']].T
        xt_er = np.ascontiguousarray(x_cat[d['dstids'].reshape(-1)].T)
        W = np.asarray(inputs['W']).astype(np.float32)
        attn_l, attn_r = np.asarray(inputs['attn_l']), np.asarray(inputs['attn_r'])
        wcat = W.transpose(1, 0, 2).reshape(P, P)
        wT = W.transpose(0, 2, 1).reshape(P, P)
        attn = np.zeros((P, 2), np.float32)
        attn[0:64, 0] = attn_l[0]; attn[64:128, 0] = attn_l[1]
        attn[0:64, 1] = attn_r[0]; attn[64:128, 1] = attn_r[1]
        in_maps.append({
            'xt': xt.astype(bf16),
            'xt_er': xt_er.astype(bf16),
            'wcat': wcat.astype(bf16),
            'wT': wT.astype(np.float32),
            'attn': attn.astype(np.float32),
            'iota': np.broadcast_to(np.arange(P, dtype=np.float32), (P, P)).astype(bf16),
            'src_g': to_grp(d['srcix'], np.int32),
            'rel_g': to_grp(d['relflat'], bf16),
            'relT': to_relT(d['relflat']),
            'iota_p': np.arange(P, dtype=np.float32).reshape(P, 1).astype(bf16),
            'ones': np.ones((1, P), np.float32),
        })

    n_ir = sum(1 for d in per_core_inst[0] if d['is_ir'])
    ir_segs = [ETYPES[d['etype']][0] for d in per_core_inst[0] if d['is_ir']]
    meta = dict(NCpad=NCpad, NSLOT=NSLOT, slot_Ts=slot_Ts, n_tiles=n_tiles,
                n_grp=n_grp, G=g, out_rows=out_rows, seg_off=seg_off,
                sizes=sizes, n_ir=n_ir, ir_segs=ir_segs)
    return in_maps, cores, meta


def build_program(meta):
    NCpad, NSLOT = meta['NCpad'], meta['NSLOT']
    slot_Ts, n_grp, G = meta['slot_Ts'], meta['n_grp'], meta['G']
    n_tiles = meta['n_tiles']
    f32, bf, i32 = mybir.dt.float32, mybir.dt.bfloat16, mybir.dt.int32
    AF = mybir.ActivationFunctionType
    OP = mybir.AluOpType

    nc = bacc.Bacc("TRN2", target_bir_lowering=False, debug=False,
                   num_devices=N_CORES)
    xt_ap = nc.dram_tensor("xt", [P, NCpad], bf, kind="ExternalInput").ap()
    xter_ap = nc.dram_tensor("xt_er", [P, NSLOT * P], bf, kind="ExternalInput").ap()
    wcat_ap = nc.dram_tensor("wcat", [P, P], bf, kind="ExternalInput").ap()
    wT_ap = nc.dram_tensor("wT", [P, P], f32, kind="ExternalInput").ap()
    attn_ap = nc.dram_tensor("attn", [P, 2], f32, kind="ExternalInput").ap()
    iota_ap = nc.dram_tensor("iota", [P, P], bf, kind="ExternalInput").ap()
    srcg_ap = nc.dram_tensor("src_g", [n_grp, P, G], i32, kind="ExternalInput").ap()
    relg_ap = nc.dram_tensor("rel_g", [n_grp, P, G], bf, kind="ExternalInput").ap()
    relT_ap = nc.dram_tensor("relT", [n_grp, 1, G * P], f32, kind="ExternalInput").ap()
    iop_ap = nc.dram_tensor("iota_p", [P, 1], bf, kind="ExternalInput").ap()
    ones_ap = nc.dram_tensor("ones", [1, P], f32, kind="ExternalInput").ap()
    stag_ap = nc.dram_tensor("stag", [NSLOT * P, P], f32, kind="ExternalOutput").ap()
    n_ir = meta['n_ir']
    stagir_ap = nc.dram_tensor("stag_ir", [max(n_ir, 1) * P, P], f32,
                               kind="ExternalOutput").ap()

    ztab = nc.dram_tensor("ztab", [NCpad, ROW], bf).ap()
    ir_ins = [nc.dram_tensor(f"ir_in{k}", [P, 130], f32) for k in range(n_ir)]
    ir_outs = [nc.dram_tensor(f"ir_out{k}", [P, 130], f32, addr_space="Shared")
               for k in range(n_ir)]

    with tile.TileContext(nc) as tc:
        with tc.tile_pool(name="cst", bufs=1) as cst, \
             tc.tile_pool(name="pa", bufs=3) as pa, \
             tc.tile_pool(name="ps_a", bufs=2, space="PSUM") as ps_a, \
             tc.tile_pool(name="sb", bufs=3) as sb, \
             tc.tile_pool(name="fl", bufs=4) as fl, \
             tc.tile_pool(name="ps", bufs=3, space="PSUM") as ps, \
             tc.tile_pool(name="psr", bufs=2, space="PSUM") as psr, \
             tc.tile_pool(name="pse", bufs=1, space="PSUM") as pse:
            # constants / weight prep
            iota_t = cst.tile([P, P], bf)
            nc.sync.dma_start(iota_t[:], iota_ap[:, :])
            iop_t = cst.tile([P, 1], bf)
            nc.sync.dma_start(iop_t[:], iop_ap[:, :])
            ones_t = cst.tile([1, P], f32)
            nc.sync.dma_start(ones_t[:], ones_ap[:, :])
            wT_t = cst.tile([P, P], f32)
            nc.sync.dma_start(wT_t[:], wT_ap[:, :])
            attn_t = cst.tile([P, 2], f32)
            nc.sync.dma_start(attn_t[:], attn_ap[:, :])
            rhs_ext = cst.tile([P, ROW], bf)
            nc.sync.dma_start(rhs_ext[:, 0:P], wcat_ap[:, :])
            for h in range(2):
                for ci in range(2):
                    wl_ps = ps_a.tile([P, 1], f32, tag="zps")
                    nc.tensor.matmul(wl_ps[:], lhsT=wT_t[h * 64:(h + 1) * 64, :],
                                     rhs=attn_t[h * 64:(h + 1) * 64, ci:ci + 1],
                                     start=True, stop=True)
                    nc.vector.tensor_copy(
                        out=rhs_ext[:, P + 2 * ci + h:P + 2 * ci + h + 1],
                        in_=wl_ps[:])
            # phase A: compact nodes -> ztab (4 chunks per DMA)
            ztab_v = ztab.rearrange("(c p) r -> p c r", p=P)
            for cb in range(NCpad // (4 * P)):
                xc = pa.tile([P, 4 * P], bf, tag="xc")
                nc.sync.dma_start(xc[:], xt_ap[:, cb * 4 * P:(cb + 1) * 4 * P])
                zrow = pa.tile([P, 4, ROW], bf, tag="zrow")
                for j in range(4):
                    zps = ps_a.tile([P, ROW], f32, tag="zps")
                    nc.tensor.matmul(zps[:], lhsT=xc[:, j * P:(j + 1) * P],
                                     rhs=rhs_ext[:], start=True, stop=True)
                    nc.vector.tensor_copy(out=zrow[:, j, 0:P], in_=zps[:, 0:P])
                    nc.vector.tensor_copy(out=zrow[:, j, P:P + 4].bitcast(f32),
                                          in_=zps[:, P:P + 2])
                nc.sync.dma_start(ztab_v[:, cb * 4:(cb + 1) * 4], zrow[:])
            # phase A: er region -> erall SBUF (no DRAM roundtrip)
            erall = cst.tile([P, NSLOT, 2], bf)
            for ib in range(0, NSLOT, 4):
                w = min(4, NSLOT - ib)
                xc = pa.tile([P, 4 * P], bf, tag="xc")
                nc.sync.dma_start(xc[:, :w * P], xter_ap[:, ib * P:(ib + w) * P])
                for j in range(w):
                    eps = ps_a.tile([P, 2], f32, tag="zps")
                    nc.tensor.matmul(eps[:], lhsT=xc[:, j * P:(j + 1) * P],
                                     rhs=rhs_ext[:, P + 2:P + 4],
                                     start=True, stop=True)
                    nc.vector.tensor_copy(out=erall[:, ib + j], in_=eps[:])

            tc.strict_bb_all_engine_barrier()

            # phase B
            slot_of_tile = np.repeat(np.arange(NSLOT), slot_Ts)
            tile_off = np.concatenate([[0], np.cumsum(slot_Ts)])
            psum_cur = None
            for grp in range(n_grp):
                t0 = grp * G
                gl = min(G, n_tiles - t0)
                if gl <= 0:
                    break
                srcix = sb.tile([P, G], i32, tag="srcix")
                relf = sb.tile([P, G], bf, tag="relf")
                relT_t = sb.tile([1, G * P], f32, tag="relT")
                nc.sync.dma_start(srcix[:], srcg_ap[grp])
                nc.sync.dma_start(relf[:], relg_ap[grp])
                nc.sync.dma_start(relT_t[:], relT_ap[grp])
                zg = sb.tile([P, G, ROW], bf, tag="zg")
                erg = sb.tile([P, G, 2], f32, tag="erg")
                for gi in range(gl):
                    nc.gpsimd.indirect_dma_start(
                        out=zg[:, gi], out_offset=None, in_=ztab[:, :],
                        in_offset=bass.IndirectOffsetOnAxis(
                            ap=srcix[:, gi:gi + 1], axis=0))
                # replicate relT across partitions (ones-matmul), cast to bf16
                relrep = sb.tile([P, G * P], bf, tag="relrep")
                for k in range((G * P + 511) // 512):
                    c0, c1 = k * 512, min((k + 1) * 512, G * P)
                    rps = psr.tile([P, 512], f32, tag="rps")
                    nc.tensor.matmul(rps[:, :c1 - c0], lhsT=ones_t[:],
                                     rhs=relT_t[:, c0:c1], start=True, stop=True)
                    nc.scalar.activation(relrep[:, c0:c1], rps[:, :c1 - c0],
                                         AF.Copy)
                S_T = sb.tile([P, G * P], bf, tag="S_T")
                nc.vector.tensor_tensor(
                    out=S_T[:],
                    in0=iop_t[:].to_broadcast([P, G * P]),
                    in1=relrep[:],
                    op=OP.is_equal)
                for gi in range(gl):
                    t_ = t0 + gi
                    s = int(slot_of_tile[t_])
                    eps_t = pse.tile([P, 2], f32, tag="eps")
                    nc.tensor.matmul(eps_t[:], lhsT=S_T[:, gi * P:(gi + 1) * P],
                                     rhs=erall[:, s], start=True, stop=True)
                    nc.vector.tensor_copy(out=erg[:, gi], in_=eps_t[:])
                S = sb.tile([P, G, P], bf, tag="S")
                nc.vector.tensor_tensor(
                    out=S[:],
                    in0=relf[:].unsqueeze(2).to_broadcast([P, G, P]),
                    in1=iota_t[:].unsqueeze(1).to_broadcast([P, G, P]),
                    op=OP.is_equal)
                e_t = sb.tile([P, G, 2], f32, tag="e")
                nc.vector.tensor_tensor(out=e_t[:],
                                        in0=zg[:].bitcast(f32)[:, :, 64:66],
                                        in1=erg[:], op=OP.add)
                lk = sb.tile([P, G, 2], f32, tag="lk")
                nc.vector.tensor_scalar(out=lk[:], in0=e_t[:], scalar1=0.2,
                                        scalar2=None, op0=OP.mult)
                nc.vector.tensor_tensor(out=lk[:], in0=lk[:], in1=e_t[:],
                                        op=OP.max)
                ee = sb.tile([P, G, 2], bf, tag="ee")
                nc.scalar.activation(ee[:], lk[:], AF.Exp)
                msg = sb.tile([P, G, 130], bf, tag="msg")
                nc.vector.tensor_tensor(
                    out=msg[:, :, 0:P].rearrange("p g (h j) -> p g h j", h=2),
                    in0=zg[:, :, 0:P].rearrange("p g (h j) -> p g h j", h=2),
                    in1=ee[:].unsqueeze(3).to_broadcast([P, G, 2, 64]),
                    op=OP.mult)
                nc.vector.tensor_copy(out=msg[:, :, P:P + 2], in_=ee[:])
                for gi in range(gl):
                    t_ = t0 + gi
                    s = int(slot_of_tile[t_])
                    tt = t_ - int(tile_off[s])
                    T = slot_Ts[s]
                    if tt == 0:
                        psum_cur = ps.tile([P, 130], f32, tag="acc")
                    nc.tensor.matmul(psum_cur[:], lhsT=S[:, gi], rhs=msg[:, gi],
                                     start=(tt == 0), stop=(tt == T - 1))
                    if tt == T - 1:
                        if s < n_ir:
                            raw = fl.tile([P, 130], f32, tag="raw")
                            nc.vector.tensor_copy(out=raw[:], in_=psum_cur[:])
                            nc.sync.dma_start(ir_ins[s].ap()[:, :], raw[:])
                            nc.gpsimd.collective_compute(
                                "AllReduce", OP.add,
                                replica_groups=[list(range(N_CORES))],
                                ins=[ir_ins[s].ap().opt()],
                                outs=[ir_outs[s].ap().opt()])
                        else:
                            den = fl.tile([P, 2], f32, tag="den")
                            nc.vector.tensor_scalar(
                                out=den[:], in0=psum_cur[:, P:P + 2],
                                scalar1=1e-9, scalar2=None, op0=OP.add)
                            rec = fl.tile([P, 2], f32, tag="rec")
                            nc.vector.reciprocal(rec[:], den[:])
                            ot = fl.tile([P, P], f32, tag="ot")
                            for h in range(2):
                                nc.scalar.activation(
                                    ot[:, h * 64:(h + 1) * 64],
                                    psum_cur[:, h * 64:(h + 1) * 64],
                                    AF.Copy, scale=rec[:, h:h + 1])
                            nc.sync.dma_start(stag_ap[s * P:(s + 1) * P, :], ot[:])
            # ir tails: load AllReduced partials, normalize
            for k in range(n_ir):
                irt = fl.tile([P, 130], f32, tag="irt")
                nc.sync.dma_start(irt[:], ir_outs[k].ap()[:, :])
                den = fl.tile([P, 2], f32, tag="den")
                nc.vector.tensor_scalar(out=den[:], in0=irt[:, P:P + 2],
                                        scalar1=1e-9, scalar2=None, op0=OP.add)
                rec = fl.tile([P, 2], f32, tag="rec")
                nc.vector.reciprocal(rec[:], den[:])
                ot = fl.tile([P, P], f32, tag="ot")
                for h in range(2):
                    nc.scalar.activation(ot[:, h * 64:(h + 1) * 64],
                                         irt[:, h * 64:(h + 1) * 64],
                                         AF.Copy, scale=rec[:, h:h + 1])
                nc.sync.dma_start(stagir_ap[k * P:(k + 1) * P, :], ot[:])
    nc.compile()
    return nc


def assemble(results, cores, meta):
    out = np.zeros((meta['out_rows'], P), np.float32)
    sizes, seg_off = meta['sizes'], meta['seg_off']
    for c in range(N_CORES):
        stag = results[c]['stag']
        for s, inst in enumerate(cores[c]['insts']):
            if inst['is_ir'] or inst['block'] < 0:
                continue
            sk, dk, st, dt = ETYPES[inst['etype']]
            base = seg_off[sk] + inst['block'] * P
            n = min(P, seg_off[sk] + sizes[dt] - base)
            out[base:base + n] = stag[s * P:s * P + n]
    # split (single-dst-block) etypes from core 0's stag_ir
    for k, sk in enumerate(meta['ir_segs']):
        dt = [e[3] for e in ETYPES if e[0] == sk][0]
        out[seg_off[sk]:seg_off[sk] + sizes[dt]] = \
            results[0]['stag_ir'][k * P:k * P + sizes[dt]]
    return out


LAST_EXEC_NS = None
LAST_PROFILE = None


def kernel(**inputs):
    global LAST_EXEC_NS, LAST_PROFILE
    in_maps, cores, meta = host_prep(inputs)
    nc = build_program(meta)
    trace = os.environ.get('KERNEL_TRACE', '0') == '1'
    res = run_bass_kernel_spmd(nc, in_maps, core_ids=list(range(N_CORES)),
                               trace=trace)
    LAST_EXEC_NS = res.exec_time_ns
    LAST_PROFILE = res.profile_json
    return assemble(res.results, cores, meta)
